# revision 5
# baseline (speedup 1.0000x reference)
"""Trainium2 Bass kernel v2: 3-layer LSTM decoder, layer-PIPELINED across cores.

vs the data-parallel baseline (each core: B=2, 3 layers serially = 3000
sequential weight-stream steps of 7.68us): here each LSTM layer lives on its
own core with the FULL batch (B=16), and chunks of Tc time steps flow through
a core pipeline via per-chunk AllGathers.  The recurrence weight-stream floor
(144 W_hh tiles x 128 cols / 2.4 GHz = 7.68us/step, batch-independent) is
paid once per layer in PARALLEL across cores instead of 3x serially:

    wall ~= (T/Tc + 6) x (Tc x 7.68us + phaseA + misc)  ~= 10 ms  (vs 23 ms)

SPMD mapping (all 8 cores run the same program; roles differ only via
per-core INPUT tensors -- no control flow):
  core 0: LSTM layer 1, input selected from its LOCAL stream (x ++ prenet)
  core 1: LSTM layer 2 + residual, input = core 0's chunk via AllGather
  core 2: LSTM layer 3 + residual + projection, input = core 1's chunk
  cores 3-7: garbage replicas (zero weights), outputs ignored.
Each iteration every core reads its predecessor's chunk from the AllGather
of iteration j-2 (the 2-iteration lag hides the ~48us AG latency), computes
its layer, and AGs its own output chunk.  The local-vs-AG input choice and
the residual are per-core 0/1 coefficient columns; state is zeroed via a
per-core per-iteration mask column until the stage's first real chunk
arrives.  Numerics identical to the baseline (bf16 weights/streams, fp32
cell state, all gate nonlinearities a single Sigmoid via the 2x g-row
prescale trick).
"""

import numpy as np
import ml_dtypes

# ---------------------------------------------------------------- constants
B, T, DX, DM = 16, 1000, 512, 128
H = 768
P = 128
HK = H // P            # 6 hidden-dim k-chunks
G = 4 * H // P         # 24 gate m-tiles
NCORES = 8
BB = B                 # full batch per pipeline stage
TC = 10                # recurrence steps per chunk (even)
CB = TC * BB           # tokens per chunk (160)
LAG = 2                # iterations between AG production and consumption
UNROLL = 2             # recurrence steps per For_i body
PIPE = [0, 1, 2]       # stage order: L1 (reads local input directly), L2, L3
NT = CB                # token tile for prenet (= CB so PSUM tag shapes match)

BF16 = ml_dtypes.bfloat16
FP8 = ml_dtypes.float8_e4m3
FP8_REC = False     # fp8 W_hh (x64-normalized) + fp8 h for the recurrence matmuls
FP8_SCALE = 64.0    # shifts N(0,1/sqrt(768)) weights out of e4m3 subnormals
SKIP_AG = False     # diag: skip collectives (wrong results; in_st <- loc chunk)
SKIP_REC = False    # diag: skip recurrence For_i (wrong results)


def _derived(t_steps):
    nc_ = t_steps // TC
    iters = nc_ + 2 * LAG
    toks = t_steps * BB
    return nc_, iters, toks


# g-gate rows pre-scaled by 2 host-side: tanh(x) = 2*sigmoid(2x) - 1.
_GSCALE = np.ones(4 * H, np.float32)
_GSCALE[2 * H:3 * H] = 2.0


# ---------------------------------------------------------------- host prep
def _prep_lhsT(w, dtype=None):
    """[M, K] weight -> stationary-operand layout [128, K/128, M]."""
    M, K = w.shape
    return np.ascontiguousarray(
        w.T.reshape(K // P, P, M).transpose(1, 0, 2)
    ).astype(dtype or BF16)


def _prep_pvec(v):
    """[N] vector -> [128, N/128] (fp32), column n = rows n*128..+128."""
    return np.ascontiguousarray(v.reshape(-1, P).T).astype(np.float32)


def _prep_inputs(inputs, t_steps=T):
    nc_, iters, toks = _derived(t_steps)
    f32 = np.float32

    shared = {}
    shared["pw1T"] = np.ascontiguousarray(
        np.asarray(inputs["pw1"]).T).astype(BF16)              # [128, 256]
    shared["pw2T"] = _prep_lhsT(np.asarray(inputs["pw2"]))     # [128, 2, 256]
    pb = np.concatenate([
        _prep_pvec(np.asarray(inputs["pb1"])),
        _prep_pvec(np.asarray(inputs["pb2"])),
    ], axis=1)
    shared["pb"] = np.ascontiguousarray(pb).astype(f32)        # [128, 4]
    shared["projT"] = _prep_lhsT(np.asarray(inputs["proj_w"])).reshape(P, HK, P)

    x = np.asarray(inputs["x"])[:, :t_steps]        # [16, T, 512]
    mels = np.asarray(inputs["mels"])[:, :t_steps]  # [16, T, 128]
    # token index = t*BB + b; feature-major
    shared["xT"] = np.ascontiguousarray(
        x.transpose(2, 1, 0).reshape(DX, toks)
        .reshape(DX // P, P, toks).transpose(1, 0, 2)).astype(BF16)  # [128,4,toks]
    shared["melsT"] = np.ascontiguousarray(
        mels.transpose(2, 1, 0).reshape(DM, toks)).astype(BF16)      # [128, toks]

    wh_dt = FP8 if FP8_REC else BF16
    wh_sc = FP8_SCALE if FP8_REC else 1.0
    zero_w = {
        "wih": np.zeros((P, HK, 4 * H), BF16),
        "whh": np.zeros((P, HK, 4 * H), wh_dt),
        "bias": np.zeros((P, G), f32),
    }
    layer_w = []
    for li in (1, 2, 3):
        wih = np.asarray(inputs[f"w_ih{li}"]) * _GSCALE[:, None]
        whh = np.asarray(inputs[f"w_hh{li}"]) * _GSCALE[:, None] * wh_sc
        bias = (np.asarray(inputs[f"b_ih{li}"]) + np.asarray(inputs[f"b_hh{li}"])) * _GSCALE
        layer_w.append({
            "wih": _prep_lhsT(wih),
            "whh": _prep_lhsT(whh, wh_dt),
            "bias": _prep_pvec(bias),
        })

    per_core = []
    for c in range(NCORES):
        stage = PIPE.index(c) if c in PIPE else None
        d = dict(layer_w[stage]) if stage is not None else dict(zero_w)
        selloc = 1.0 if stage == 0 else 0.0
        selag = 0.0 if stage == 0 else 1.0
        res = 1.0 if (stage in (1, 2)) else 0.0
        cfg = np.zeros((P, 4), f32)
        cfg[:, 0], cfg[:, 1], cfg[:, 2] = selloc, selag, res
        d["cfg"] = cfg
        first_real = stage * LAG if stage is not None else iters
        sm = np.ones((P, iters), f32)
        sm[:, :min(first_real + 1, iters)] = 0.0
        d["smask"] = sm
        per_core.append(d)
    return shared, per_core


# ---------------------------------------------------------------- bass build
def _emit(ctx, tc, d, t_steps):
    import concourse.mybir as mybir
    from concourse.bass import ds, ts

    nc_, iters, toks = _derived(t_steps)
    nc = tc.nc
    f32 = mybir.dt.float32
    bf16 = mybir.dt.bfloat16
    AF = mybir.ActivationFunctionType
    AO = mybir.AluOpType

    sbt = lambda name, shape, dt: nc.alloc_sbuf_tensor(name, list(shape), dt)

    # persistent SBUF tensors
    f8 = mybir.dt.float8e4
    wh_dt = f8 if FP8_REC else bf16
    wih_sb = sbt("wih_sb", [P, HK, 4 * H], bf16)
    whh_sb = sbt("whh_sb", [P, HK, 4 * H], wh_dt)
    bias_sb = sbt("bias_sb", [P, G], f32)
    pw1_sb = sbt("pw1_sb", [P, 2 * P], bf16)
    pw2_sb = sbt("pw2_sb", [P, 2, 2 * P], bf16)
    pb_sb = sbt("pb_sb", [P, 4], f32)
    proj_sb = sbt("proj_sb", [P, HK, P], bf16)
    cfg_sb = sbt("cfg_sb", [P, 4], f32)
    smask_sb = sbt("smask_sb", [P, iters], f32)
    hst = sbt("hst", [P, HK, BB], bf16)
    h8 = sbt("h8", [P, HK, BB], f8) if FP8_REC else hst
    cst = sbt("cst", [P, HK, BB], f32)

    tmp = ctx.enter_context(tc.tile_pool(name="tmp", bufs=2))
    stream = ctx.enter_context(tc.tile_pool(name="stream", bufs=2))
    xgp = ctx.enter_context(tc.tile_pool(name="xgp", bufs=1))
    psA = ctx.enter_context(tc.tile_pool(name="psA", bufs=2, space="PSUM"))
    psG1 = ctx.enter_context(tc.tile_pool(name="psG1", bufs=1, space="PSUM"))
    psG2 = ctx.enter_context(tc.tile_pool(name="psG2", bufs=1, space="PSUM"))
    psP = ctx.enter_context(tc.tile_pool(name="psP", bufs=1, space="PSUM"))
    dram = ctx.enter_context(tc.tile_pool(name="dram", bufs=1, space="DRAM"))

    # DRAM tensors
    loc_in = dram.tile([P, HK, toks], bf16, tag="loc_in", name="loc_in")
    ag_ins = [dram.tile([P, HK, CB], bf16, tag=f"agi{j}", name=f"agi{j}")
              for j in range(iters)]
    ag_outs = [dram.tile([NCORES * P, HK, CB], bf16, tag=f"ago{j}",
                         name=f"ago{j}", addr_space="Shared")
               for j in range(iters)]

    # ---- load constants
    nc.sync.dma_start(out=wih_sb[:], in_=d["wih"][:])
    nc.sync.dma_start(out=whh_sb[:], in_=d["whh"][:])
    nc.sync.dma_start(out=bias_sb[:], in_=d["bias"][:])
    nc.sync.dma_start(out=pw1_sb[:], in_=d["pw1T"][:])
    nc.sync.dma_start(out=pw2_sb[:], in_=d["pw2T"][:])
    nc.sync.dma_start(out=pb_sb[:], in_=d["pb"][:])
    nc.sync.dma_start(out=proj_sb[:], in_=d["projT"][:])
    nc.sync.dma_start(out=cfg_sb[:], in_=d["cfg"][:])
    nc.sync.dma_start(out=smask_sb[:], in_=d["smask"][:])
    nc.vector.memset(hst[:], 0.0)
    nc.vector.memset(cst[:], 0.0)
    if FP8_REC:
        nc.vector.memset(h8[:], 0.0)

    # ---- assemble layer-1 input stream in DRAM: loc_in = xT ++ prenet(mels)
    nc.sync.dma_start(out=loc_in[:, 0:4, :], in_=d["xT"][:])
    for i0 in range(0, toks, NT):
        mstage = tmp.tile([P, NT], bf16, tag="mstage")
        nc.sync.dma_start(out=mstage[:], in_=d["melsT"][:, i0:i0 + NT])
        m1 = tmp.tile([P, 2, NT], bf16, tag="m1")
        for mi in range(2):
            ps = psA.tile([P, NT], f32, tag="pa")
            nc.tensor.matmul(ps[:], lhsT=pw1_sb[:, ts(mi, P)],
                             rhs=mstage[:], start=True, stop=True)
            nc.scalar.activation(m1[:, mi, :], ps[:], AF.Relu,
                                 bias=pb_sb[:, mi:mi + 1], scale=1.0)
        m2 = tmp.tile([P, 2, NT], bf16, tag="m2")
        for mi in range(2):
            ps = psA.tile([P, NT], f32, tag="pa")
            for k in range(2):
                nc.tensor.matmul(ps[:], lhsT=pw2_sb[:, k, ts(mi, P)],
                                 rhs=m1[:, k, :], start=(k == 0), stop=(k == 1))
            nc.scalar.activation(m2[:, mi, :], ps[:], AF.Relu,
                                 bias=pb_sb[:, 2 + mi:3 + mi], scale=1.0)
        nc.sync.dma_start(out=loc_in[:, 4:6, i0:i0 + NT], in_=m2[:])

    pid = nc.partition_id(engines=list(mybir.ALL_ENGINES))
    prevrow = ((pid + (NCORES - 1)) % NCORES) * P

    # ---- pipeline iterations
    for j in range(iters):
        in_st = stream.tile([P, HK, CB], bf16, tag="in_st")
        if j < LAG or SKIP_AG:
            nc.vector.memset(in_st[:], 0.0)
        else:
            nc.sync.dma_start(out=in_st[:],
                              in_=ag_outs[j - LAG][ds(prevrow, P), :, :])
        lchunk = min(j, nc_ - 1)
        loc_st = stream.tile([P, HK, CB], bf16, tag="loc_st")
        nc.sync.dma_start(out=loc_st[:],
                          in_=loc_in[:, :, lchunk * CB:(lchunk + 1) * CB])

        # input select: core 0 takes its local chunk, others the AG block
        t_ag = tmp.tile([P, HK, CB], bf16, tag="t_ag")
        nc.vector.tensor_scalar(t_ag[:], in_st[:], cfg_sb[:, 1:2], None, AO.mult)
        in_eff = stream.tile([P, HK, CB], bf16, tag="in_eff")
        nc.vector.scalar_tensor_tensor(in_eff[:], loc_st[:], cfg_sb[:, 0:1],
                                       t_ag[:], AO.mult, AO.add)

        # residual-scaled input (res=0 or 1 per core)
        in_res = stream.tile([P, HK, CB], bf16, tag="in_res")
        nc.vector.tensor_scalar(in_res[:], in_eff[:], cfg_sb[:, 2:3], None, AO.mult)

        # phase A: xg = W_ih @ in + b   [P, G, CB] fp32
        xg_sb = xgp.tile([P, G, CB], f32, tag="xg")
        for m in range(G):
            ps = psA.tile([P, CB], f32, tag="pa")
            for k in range(HK):
                nc.tensor.matmul(ps[:], lhsT=wih_sb[:, k, ts(m, P)],
                                 rhs=in_eff[:, k, :],
                                 start=(k == 0), stop=(k == HK - 1))
            nc.vector.tensor_scalar(xg_sb[:, m, :], ps[:],
                                    bias_sb[:, m:m + 1], None, AO.add)

        # state mask (zero until this stage's first real chunk)
        nc.vector.tensor_scalar(hst[:], hst[:], smask_sb[:, j:j + 1], None, AO.mult)
        nc.vector.tensor_scalar(cst[:], cst[:], smask_sb[:, j:j + 1], None, AO.mult)
        if FP8_REC:
            nc.vector.tensor_scalar(h8[:], h8[:], smask_sb[:, j:j + 1], None, AO.mult)

        out_st = stream.tile([P, HK, CB], bf16, tag="out_st")

        if SKIP_REC:
            nc.vector.tensor_scalar(out_st[:], in_res[:], 1.0, None, AO.mult)
        # phase B: TC recurrence steps, 2 per hardware-loop body
        with tc.For_i(0, CB, UNROLL * BB, hint_engines=(mybir.EngineType.PE,)) as sl0:
            for u in (() if SKIP_REC else range(UNROLL)):
                sl = ds(sl0 + u * BB, BB) if u else ds(sl0, BB)
                pg1 = psG1.tile([P, 18, BB], f32, tag=f"pg1{u % 2}")
                pg2 = psG2.tile([P, HK, BB], f32, tag=f"pg2{u % 2}")
                for m in range(18):
                    for k in range(HK):
                        nc.tensor.matmul(pg1[:, m, :], lhsT=whh_sb[:, k, ts(m, P)],
                                         rhs=h8[:, k, :],
                                         start=(k == 0), stop=(k == HK - 1))
                for m in range(18, 24):
                    for k in range(HK):
                        nc.tensor.matmul(pg2[:, m - 18, :], lhsT=whh_sb[:, k, ts(m, P)],
                                         rhs=h8[:, k, :],
                                         start=(k == 0), stop=(k == HK - 1))
                g1 = tmp.tile([P, 18, BB], f32, tag=f"g1{u % 2}")
                if FP8_REC:
                    nc.vector.scalar_tensor_tensor(g1[:], pg1[:], 1.0 / FP8_SCALE,
                                                   xg_sb[:, 0:18, sl],
                                                   AO.mult, AO.add)
                else:
                    nc.vector.tensor_add(g1[:], pg1[:], xg_sb[:, 0:18, sl])
                a1 = tmp.tile([P, 18, BB], f32, tag=f"a1{u % 2}")   # sig(i,f) | sig(2g)
                nc.scalar.activation(a1[:], g1[:], AF.Sigmoid)
                tg = tmp.tile([P, HK, BB], f32, tag=f"tg{u % 2}")   # tanh(g)
                nc.vector.tensor_scalar(tg[:], a1[:, 12:18, :], 2.0, -1.0,
                                        AO.mult, AO.add)
                t1 = tmp.tile([P, HK, BB], f32, tag=f"t1{u % 2}")
                nc.vector.tensor_mul(t1[:], a1[:, 6:12, :], cst[:])
                t2 = tmp.tile([P, HK, BB], f32, tag=f"t2{u % 2}")
                nc.vector.tensor_mul(t2[:], a1[:, 0:6, :], tg[:])
                nc.vector.tensor_add(cst[:], t1[:], t2[:])      # in-place c
                a2 = tmp.tile([P, HK, BB], f32, tag=f"a2{u % 2}")   # sig(2c)
                nc.scalar.activation(a2[:], cst[:], AF.Sigmoid, scale=2.0)
                tc2 = tmp.tile([P, HK, BB], f32, tag=f"tc2{u % 2}")  # tanh(c)
                nc.vector.tensor_scalar(tc2[:], a2[:], 2.0, -1.0,
                                        AO.mult, AO.add)
                g2 = tmp.tile([P, HK, BB], f32, tag=f"g2{u % 2}")
                if FP8_REC:
                    nc.vector.scalar_tensor_tensor(g2[:], pg2[:], 1.0 / FP8_SCALE,
                                                   xg_sb[:, 18:24, sl],
                                                   AO.mult, AO.add)
                else:
                    nc.vector.tensor_add(g2[:], pg2[:], xg_sb[:, 18:24, sl])
                a3 = tmp.tile([P, HK, BB], f32, tag=f"a3{u % 2}")   # sig(o)
                nc.scalar.activation(a3[:], g2[:], AF.Sigmoid)
                if FP8_REC:
                    nc.vector.tensor_mul(h8[:], a3[:], tc2[:])  # matmul operand
                nc.vector.tensor_mul(hst[:], a3[:], tc2[:])     # in-place h
                nc.vector.tensor_add(out_st[:, :, sl], hst[:], in_res[:, :, sl])

        if not SKIP_AG:
            nc.sync.dma_start(out=ag_ins[j][:], in_=out_st[:])
            nc.gpsimd.collective_compute(
                "AllGather",
                mybir.AluOpType.bypass,
                replica_groups=[list(range(NCORES))],
                ins=[ag_ins[j][:].opt()],
                outs=[ag_outs[j][:].opt()],
            )

        # projection of the output stream (real only on the last stage core)
        slot = j - 2 * LAG if j >= 2 * LAG else nc_
        ps = psP.tile([P, CB], f32, tag="pp")
        for k in range(HK):
            nc.tensor.matmul(ps[:], lhsT=proj_sb[:, k, :], rhs=out_st[:, k, :],
                             start=(k == 0), stop=(k == HK - 1))
        y = tmp.tile([P, CB], f32, tag="y")
        nc.scalar.copy(y[:], ps[:])
        nc.sync.dma_start(out=d["yT"][:, slot * CB:(slot + 1) * CB], in_=y[:])


def build_program(t_steps=T):
    assert t_steps % TC == 0
    nc_, iters, toks = _derived(t_steps)
    import concourse.bacc as bacc
    import concourse.tile as tile
    import concourse.mybir as mybir
    from contextlib import ExitStack

    f32 = mybir.dt.float32
    bf16 = mybir.dt.bfloat16

    nc = bacc.Bacc("TRN2", debug=False, num_devices=NCORES)
    d = {
        "xT": nc.dram_tensor("xT", [P, DX // P, toks], bf16, kind="ExternalInput"),
        "melsT": nc.dram_tensor("melsT", [P, toks], bf16, kind="ExternalInput"),
        "wih": nc.dram_tensor("wih", [P, HK, 4 * H], bf16, kind="ExternalInput"),
        "whh": nc.dram_tensor("whh", [P, HK, 4 * H],
                              mybir.dt.float8e4 if FP8_REC else bf16,
                              kind="ExternalInput"),
        "bias": nc.dram_tensor("bias", [P, G], f32, kind="ExternalInput"),
        "pw1T": nc.dram_tensor("pw1T", [P, 2 * P], bf16, kind="ExternalInput"),
        "pw2T": nc.dram_tensor("pw2T", [P, 2, 2 * P], bf16, kind="ExternalInput"),
        "pb": nc.dram_tensor("pb", [P, 4], f32, kind="ExternalInput"),
        "projT": nc.dram_tensor("projT", [P, HK, P], bf16, kind="ExternalInput"),
        "cfg": nc.dram_tensor("cfg", [P, 4], f32, kind="ExternalInput"),
        "smask": nc.dram_tensor("smask", [P, iters], f32, kind="ExternalInput"),
        "yT": nc.dram_tensor("yT", [P, (nc_ + 1) * CB], f32, kind="ExternalOutput"),
    }
    with tile.TileContext(nc) as tc:
        with ExitStack() as ctx:
            _emit(ctx, tc, d, t_steps)
    nc.compile()
    return nc


# ---------------------------------------------------------------- entry point
_CACHE = {}


def kernel(**inputs):
    from concourse.bass_utils import run_bass_kernel_spmd

    shared, per_core = _prep_inputs(inputs, T)

    if "nc" not in _CACHE:
        _CACHE["nc"] = build_program(T)
    nc = _CACHE["nc"]

    in_maps = [{**shared, **pc} for pc in per_core]
    res = run_bass_kernel_spmd(nc, in_maps, core_ids=list(range(NCORES)))
    _CACHE["last_res"] = res

    nc_, _, _ = _derived(T)
    yT = res.results[PIPE[-1]]["yT"][:, :nc_ * CB]          # [128, T*BB]
    out = yT.reshape(P, nc_, TC, BB).transpose(3, 1, 2, 0).reshape(BB, T, P)
    return np.ascontiguousarray(out)
